# revision 29
# baseline (speedup 1.0000x reference)
"""Trainium2 Bass kernel for CropConv (stride-2 3x3 conv, B=32 CIN=COUT=256,
H=W=64 -> 32x32; the crop mask is provably all-ones so output == conv output).

Strategy: data-parallel over batch across 8 NeuronCores (4 images/core).
Host pads each image to 66x66 and splits it into 4 polyphase components
(row/col parity), so each conv tap's moving operand is a fully contiguous
window. Per core the conv is 18 accumulated matmuls per 512-position output
tile: 9 taps x 2 cin-128-chunks, contracting cin on the PE partition dim.
PSUM accumulates fp32. Matmul operands are fp16 (11-bit mantissa; data is
unit-scale so no range issues) giving 1 cycle/row PE throughput with
~3e-4 relative error vs the fp32 reference.

Schedule (from NTFF trace analysis; see trace notes in the session):
- All DMAs ride the two HWDGE queues (sync + scalar). gpsimd's SWDGE queue
  moves data at ~12 GB/s and adds a ~2.2us dge-drain at kernel end, so it
  is not used at all.
- The PE starts duty-limited; a fixed ~68us full-clock window opens ~3-6us
  after CONTINUOUS PE activity begins, and a >1.5us PE gap revokes it.
  Ten 512-row warmup matmuls keep the PE busy from ~7.1us until real data
  flows steadily (~11.8us), guaranteeing a gap-free window entry.
- Taps are ordered phase-sorted (TAP_ORDER) so each x tile's polyphase
  pieces are consumed in DMA arrival order; the host permutes the w column
  layout to match, so w JIT slices are sequential too. Startup pieces are
  deadline-ordered across the two queues.
- Output is written as bf16 (halves output DMA bytes; adds ~2e-3 rel err
  against a 2e-2 budget). The final tile drains in parallel: the vector
  engine casts half 0 (DMA via sync) while the scalar engine casts half 1
  (DMA via scalar).
- An fp8e4 DoubleRow variant ("f16x8") was measured NOT faster (a DR
  matmul costs the same as the two fp16 matmuls it replaces) and is kept
  only for reference.
"""

import numpy as np

import concourse.bacc as bacc
import concourse.mybir as mybir
import concourse.tile as tile
from concourse.bass_utils import run_bass_kernel_spmd

B, CIN, COUT, H, W = 32, 256, 256, 64, 64
OH, OW = 32, 32
NCORES = 8
BL = B // NCORES          # images per core
KC = CIN // 128           # cin chunks
MC = COUT // 128          # cout chunks
NT = 2                    # output row-halves per image (16 rows x 32 cols = 512)
RT = OH // NT             # out rows per tile
PR = 17                   # phase rows per half (16 + 1 halo)
PC = 33                   # phase cols
PH = PR * PC              # free size of one phase (561)
XHALF = 4 * PH            # free size of one x half-tile

TAPS = [(ky, kx) for ky in range(3) for kx in range(3)]
# Phase-sorted tap order: phases are consumed 0,0,0,0,1,1,2,2,3 so each x
# tile's phase pieces are needed in DMA arrival order, maximizing just-in-
# time slack during startup. The host permutes the w layout to match, so w
# slices are also consumed sequentially.
TAP_ORDER = [(0, 0), (0, 2), (2, 0), (2, 2), (0, 1), (2, 1), (1, 0), (1, 2),
             (1, 1)]

_CACHE = {}


def _build(mm_dtype="f16", n_warm=10, warm_mv=512):
    # "f16x8": fp16 path but tap (0,0) (both cin chunks) is computed by a
    # single fp8e4 DoubleRow matmul (contracts 256 per pass), replacing two
    # fp16 matmuls. Max rel err measured 1.31e-2 vs the 2e-2 budget.
    fp8_tap = mm_dtype == "f16x8"
    io_dt = {
        "bf16": mybir.dt.bfloat16,
        "f16": mybir.dt.float16,
        "f16x8": mybir.dt.float16,
        "f32r": mybir.dt.float32r,
        "f32": mybir.dt.float32,
    }[mm_dtype]
    nc = bacc.Bacc("TRN2", target_bir_lowering=False, debug=False,
                   num_devices=NCORES)
    x = nc.dram_tensor("x", [BL, KC, NT, 128, XHALF], io_dt, kind="ExternalInput")
    w = nc.dram_tensor("w", [MC, 128, 9 * KC * 128], io_dt, kind="ExternalInput")
    y = nc.dram_tensor("y", [BL, MC, 128, OH * OW], mybir.dt.bfloat16,
                       kind="ExternalOutput")
    if fp8_tap:
        x8 = nc.dram_tensor("x8", [BL, NT, 128, KC * RT * OW],
                            mybir.dt.float8e4, kind="ExternalInput")
        w8 = nc.dram_tensor("w8", [MC, 128, KC * 128], mybir.dt.float8e4,
                            kind="ExternalInput")

    with tile.TileContext(nc) as tc:
        with (
            tc.tile_pool(name="wpool", bufs=MC) as wpool,
            tc.tile_pool(name="xpool", bufs=BL * KC * NT) as xpool,
            tc.tile_pool(name="opool", bufs=12) as opool,
            tc.tile_pool(name="spool", bufs=1) as spool,
            tc.tile_pool(name="psum", bufs=8, space="PSUM") as psum_pool,
        ):
            # PE warm-up: matmuls on a zeroed scratch tile keep the PE
            # CONTINUOUSLY active from ~7us until real data flows steadily
            # (~11.3us). The HAM full-clock window opens ~3us after the start
            # of sustained PE activity; any idle gap before that delays it
            # (measured: gappy warmups pushed HAM from 10.6us to 17.4us).
            if n_warm:
                scratch = spool.tile([128, 128 + warm_mv], io_dt)
                nc.vector.memset(scratch[:], 0.0)
                wps = psum_pool.tile([128, warm_mv], mybir.dt.float32,
                                     name="warm_ps", tag="ps")
                for _ in range(n_warm):
                    nc.tensor.matmul(wps[:], scratch[:, :128],
                                     scratch[:, 128:], start=True, stop=True)

            w_sb = {}
            x_sb = {}
            trig = [nc.sync, nc.scalar]
            n_trig = 0

            def next_eng():
                nonlocal n_trig
                e = trig[n_trig % 2]
                n_trig += 1
                return e

            w_sb[0] = wpool.tile([128, 9 * KC * 128], io_dt, tag="wsb", name="wsb0")
            w_sb[1] = wpool.tile([128, 9 * KC * 128], io_dt, tag="wsb", name="wsb1")
            for b in range(BL):
                for nt in range(NT):
                    for kc in range(KC):
                        x_sb[(b, kc, nt)] = xpool.tile(
                            [128, XHALF], io_dt, tag="ximg",
                            name=f"x_{b}_{kc}_{nt}")
            x8_sb = {}
            w8_sb = {}
            if fp8_tap:
                for b in range(BL):
                    for nt in range(NT):
                        x8_sb[(b, nt)] = xpool.tile(
                            [128, KC * RT * OW], mybir.dt.float8e4,
                            tag="ximg8", name=f"x8_{b}_{nt}")
                for mc in range(MC):
                    w8_sb[mc] = wpool.tile([128, KC * 128], mybir.dt.float8e4,
                                           tag="wsb8", name=f"w8sb{mc}")

            def w_piece(mc, c0, c1):
                next_eng().dma_start(w_sb[mc][:, c0:c1], w.ap()[mc][:, c0:c1])

            def x_piece(b, kc, nt, c0, c1):
                next_eng().dma_start(x_sb[(b, kc, nt)][:, c0:c1],
                                     x.ap()[b, kc, nt, :, c0:c1])

            def x8_piece(b, nt):
                next_eng().dma_start(x8_sb[(b, nt)][:], x8.ap()[b, nt])

            def w8_piece(mc):
                next_eng().dma_start(w8_sb[mc][:], w8.ap()[mc])

            # Startup pieces in PE consumption order. The first matmul group
            # (b0, nt0, mc0) walks w_sb[0] cols 0..2304 (kc-major, tap-minor)
            # and x(0,kc,0) phases in order 0,1,0,2,3,2,0,1,0.
            # Startup pieces sized to the ~230 GB/s combined early delivery
            # rate of the two HW queues: only w0 is needed in pass 1 (mc is
            # the outer compute loop), and w0's kc0 taps arrive as three
            # 3-tap slices just-in-time. w1 is deliberately LAST: its
            # deadline is pass 2 (~44us).
            # t0k: first fp16 tap column per kc block (tap 0 rides the fp8
            # path in f16x8 mode, so its fp16 weights are never fetched).
            t0k = 1 if fp8_tap else 0
            if fp8_tap:
                w8_piece(0)
                x8_piece(0, 0)
            w_piece(0, t0k * 128, 3 * 128)    # kc0 taps .-2
            x_piece(0, 0, 0, 0, PH)           # phase 0
            x_piece(0, 0, 0, PH, 2 * PH)      # phase 1
            w_piece(0, 3 * 128, 6 * 128)      # kc0 taps 3-5
            x_piece(0, 0, 0, 2 * PH, 3 * PH)  # phase 2
            x_piece(0, 0, 0, 3 * PH, XHALF)   # phase 3
            w_piece(0, 6 * 128, 9 * 128)      # kc0 taps 6-8
            x_piece(0, 1, 0, 0, PH)           # kc1 phase 0
            x_piece(0, 1, 0, PH, 2 * PH)      # kc1 phase 1
            w_piece(0, (9 + t0k) * 128, 12 * 128)   # kc1 taps .-2
            x_piece(0, 1, 0, 2 * PH, 3 * PH)  # kc1 phase 2
            x_piece(0, 1, 0, 3 * PH, XHALF)   # kc1 phase 3
            w_piece(0, 12 * 128, 15 * 128)    # kc1 taps 3-5
            w_piece(0, 15 * 128, 18 * 128)    # kc1 taps 6-8
            if fp8_tap:
                w8_piece(1)
            w_piece(1, t0k * 128, 4 * 128)    # mc1 kc0 (deadline ~mm18)
            w_piece(1, 4 * 128, 9 * 128)
            w_piece(1, (9 + t0k) * 128, 13 * 128)  # mc1 kc1 (~mm27)
            w_piece(1, 13 * 128, 18 * 128)
            if fp8_tap:
                x8_piece(0, 1)
            x_piece(0, 0, 1, 0, 2 * PH)
            x_piece(0, 0, 1, 2 * PH, XHALF)
            x_piece(0, 1, 1, 0, 2 * PH)
            x_piece(0, 1, 1, 2 * PH, XHALF)
            for b in range(1, BL):
                if fp8_tap:
                    x8_piece(b, 0)
                    x8_piece(b, 1)
                for nt in range(NT):
                    for kc in range(KC):
                        x_piece(b, kc, nt, 0, XHALF)

            n_groups = BL * NT
            i_group = 0
            for b in range(BL):
                for nt in range(NT):
                    i_group += 1
                    for mc in range(MC):
                        ps = psum_pool.tile([128, RT * OW], mybir.dt.float32,
                                            name=f"ps_{b}_{mc}_{nt}", tag="ps")
                        taps = ([t for t in TAP_ORDER if t != (0, 0)]
                                if fp8_tap else TAP_ORDER)
                        n_mm = KC * len(taps) + (1 if fp8_tap else 0)
                        i_mm = 0
                        if fp8_tap:
                            lhsT8 = w8_sb[mc][:].rearrange(
                                "p (two m) -> p two m", two=2)
                            rhs8 = x8_sb[(b, nt)][:].rearrange(
                                "p (two n) -> p two n", two=2)
                            nc.tensor.matmul(
                                ps[:], lhsT8, rhs8, start=True, stop=False,
                                perf_mode=mybir.MatmulPerfMode.DoubleRow,
                            )
                            i_mm += 1
                        for kc in range(KC):
                            xv = x_sb[(b, kc, nt)][:].rearrange(
                                "p (ph r c) -> p ph r c", ph=4, c=PC)
                            for t_i, (ky, kx) in enumerate(taps):
                                phase = (ky % 2) * 2 + (kx % 2)
                                r0 = ky // 2
                                c0 = kx // 2
                                # w columns are host-permuted to TAP_ORDER
                                lhsT = w_sb[mc][:, (kc * 9 + t_i + t0k)
                                                * 128:][:, :128]
                                rhs = xv[:, phase, r0:r0 + RT, c0:c0 + OW]
                                nc.tensor.matmul(
                                    ps[:], lhsT, rhs,
                                    start=(i_mm == 0), stop=(i_mm == n_mm - 1),
                                )
                                i_mm += 1
                        # One full-width output descriptor per tile: 1KB rows
                        # halve the DMA packet count vs two 256-col chunks,
                        # and 16 triggers replace 32.
                        last_tile = i_group == n_groups and mc == MC - 1
                        half = RT * OW // 2
                        ot = opool.tile([128, RT * OW], mybir.dt.bfloat16,
                                        tag="ostage")
                        ysl = y.ap()[b, mc, :, nt * 512:(nt + 1) * 512]
                        if last_tile:
                            # Final tile: cast halves in parallel on vector
                            # and scalar so the single DMA trigger fires one
                            # half-cast after the last matmul.
                            nc.vector.tensor_copy(ot[:, :half], ps[:, :half])
                            nc.scalar.activation(
                                ot[:, half:], ps[:, half:],
                                mybir.ActivationFunctionType.Copy)
                            nc.sync.dma_start(ysl, ot[:])
                        else:
                            nc.vector.tensor_copy(ot[:], ps[:])
                            next_eng().dma_start(ysl, ot[:])
    nc.compile()
    return nc


def _get(mm_dtype="f16"):
    if mm_dtype not in _CACHE:
        _CACHE[mm_dtype] = _build(mm_dtype)
    return _CACHE[mm_dtype]


def _np_dt(mm_dtype):
    if mm_dtype == "bf16":
        import ml_dtypes
        return ml_dtypes.bfloat16
    if mm_dtype in ("f16", "f16x8"):
        return np.float16
    return np.float32


def _prep_inputs(x, weight, mm_dtype="f16"):
    np_dt = _np_dt(mm_dtype)
    # x: [B, CIN, H, W] -> pad to 66x66 (top/left zero) -> 4 polyphase
    # components [pr, pc, 33, 33] -> row-halves with 1-row halo.
    xf = np.asarray(x, dtype=np.float32)
    xp = np.zeros((B, CIN, 66, 66), dtype=np_dt)
    xp[:, :, 1:1 + H, 1:1 + W] = xf
    xph = xp.reshape(B, CIN, 33, 2, 33, 2).transpose(0, 1, 3, 5, 2, 4)
    # xph: [B, CIN, pr, pc, 33, 33]
    halves = np.stack([xph[..., 0:PR, :], xph[..., 33 - PR:33, :]], axis=2)
    # halves: [B, CIN, half, pr, pc, PR, PC]
    xs = halves.reshape(NCORES, BL, KC, 128, NT, XHALF).transpose(0, 1, 2, 4, 3, 5)
    xs = np.ascontiguousarray(xs)  # [NCORES, BL, KC, NT, 128, XHALF]
    # weight: [COUT, CIN, 3, 3] -> [mc, p(cin%128), kc, tap, m(cout%128)]
    wh32 = np.asarray(weight, dtype=np.float32).transpose(2, 3, 1, 0)  # ky,kx,cin,cout
    wh32 = wh32.reshape(9, KC, 128, MC, 128)
    wh32 = wh32[[ky * 3 + kx for (ky, kx) in TAP_ORDER]]  # tap-position order
    wh32 = wh32.transpose(3, 2, 1, 0, 4)
    wh32 = np.ascontiguousarray(wh32.reshape(MC, 128, 9 * KC * 128))
    wh = wh32.astype(np_dt)
    if mm_dtype != "f16x8":
        return [{"x": xs[c], "w": wh} for c in range(NCORES)]

    # fp8 operands for tap (0,0): cast from fp32 (not fp16). x8 holds the
    # pre-windowed contiguous rhs for tap (0,0): phase-0 rows 0:16, cols
    # 0:32 per (kc, nt), flattened kc-major to [128, KC*512].
    import ml_dtypes
    e4 = ml_dtypes.float8_e4m3
    xp32 = np.zeros((B, CIN, 66, 66), dtype=np.float32)
    xp32[:, :, 1:1 + H, 1:1 + W] = xf
    ph0 = xp32[:, :, 0::2, 0::2]  # [B, CIN, 33, 33]
    win = np.stack([ph0[:, :, 0:RT, 0:OW],
                    ph0[:, :, 33 - PR:33 - PR + RT, 0:OW]], axis=2)
    # win: [B, CIN, nt, 16, 32]
    x8 = win.reshape(NCORES, BL, KC, 128, NT, RT * OW).transpose(0, 1, 4, 3, 2, 5)
    x8 = np.ascontiguousarray(
        x8.reshape(NCORES, BL, NT, 128, KC * RT * OW)).astype(e4)
    # w8: tap (0,0) columns of wh32, kc-major: [MC, 128, KC*128]
    w8 = np.stack([wh32[:, :, (kc * 9) * 128:(kc * 9 + 1) * 128]
                   for kc in range(KC)], axis=2)
    w8 = np.ascontiguousarray(
        w8.reshape(MC, 128, KC * 128)).astype(e4)
    return [{"x": xs[c], "w": wh, "x8": x8[c], "w8": w8}
            for c in range(NCORES)]


def run(x, weight, mm_dtype="f16", **spmd_kwargs):
    nc = _get(mm_dtype)
    in_maps = _prep_inputs(x, weight, mm_dtype)
    res = run_bass_kernel_spmd(nc, in_maps, core_ids=list(range(NCORES)),
                               **spmd_kwargs)
    out = np.empty((B, COUT, OH, OW), dtype=np.float32)
    for c in range(NCORES):
        yc = np.asarray(res.results[c]["y"]).astype(np.float32)
        out[c * BL:(c + 1) * BL] = yc.reshape(BL, COUT, OH, OW)
    return out, res


def kernel(x, weight):
    out, _ = run(x, weight)
    return out
